# revision 48
# baseline (speedup 1.0000x reference)
"""Distributed Trainium2 kernel: relative-position multi-head attention.

B=2, N=2048, DIM=1536, H=8, DK=64, DV=192.

Sharding: one head per core, both batches (8 heads / 8 cores).  Each core
projects q/k/v for its head over all 4096 tokens, runs attention, transposes
its attention output to feature-major, then four 8-core AllToAlls (one per
(batch, half)) exchange 128-row granules: core c ends up with all 8 heads'
outputs for rows [b*2048 + h*1024 + c*128, +128), and computes those rows of
the output projection.

Math: rel_k = distances @ W_rel is rank-1, so after relative_shift the
positional logits are s_i*(j-i) with s_i = (q_i*scale+rpb)@w_h.  The -s_i*i
term is constant per softmax row and drops under softmax.  So
logits = (q*scale+rcb)@k^T + s_i*j, realized as 3 extra contraction rows of
the QK^T matmul: [jhi, jlo, ones] on the K side and [8s, s, -B] on the Q
side (j = 8*jhi + jlo keeps the ramp exact in bf16), where B upper-bounds
the row max so exp cannot overflow; it cancels exactly in softmax.
s = w_h^T @ (q*scale + rcb) + (rpb-rcb)@w_h is computed by one extra matmul
against the already-projected QT content rows.
"""

import contextlib
import os
import sys

if os.path.isdir("/opt/trn_rl_repo") and "/opt/trn_rl_repo" not in sys.path:
    sys.path.insert(0, "/opt/trn_rl_repo")

import ml_dtypes
import numpy as np

import concourse.bass as bass
import concourse.bacc as bacc_mod
import concourse.mybir as mybir
import concourse.tile as tile
from concourse.bass_utils import run_bass_kernel_spmd
from concourse.masks import make_identity

B, N, DIM, H, DK, DV = 2, 2048, 1536, 8, 64, 192
NCORES = 8
NT = B * N               # 4096 flat tokens
NQ = NT // NCORES        # 512 output rows per core
P = 128
DCH = DIM // P           # 12 projection contraction chunks
NTILE = N // P           # 16 token tiles per batch
IBLK = 512
NIB = N // IBLK          # 4 i-blocks per batch
GRAN = 128               # output-row granule per core per (batch, half)
F32 = mybir.dt.float32
BF16 = mybir.dt.bfloat16
AT = mybir.AluOpType
AF = mybir.ActivationFunctionType
CONTENT_BOUND = 48.0

_CACHE = {}


def _build_body(nc, tc, xT, wqk, wv, wo, krows, ccol, bor, out):
    ctx = contextlib.ExitStack()
    with ctx:
        persist = ctx.enter_context(tc.tile_pool(name="persist", bufs=1))

        wqk_sb = persist.tile([P, DCH * P], BF16, tag="wqk")
        wv_sb = persist.tile([P, DCH * (DV + 1)], BF16, tag="wv")
        wo_sb = persist.tile([P, DCH * DIM], BF16, tag="wo")
        ccol_sb = persist.tile([P, 2], F32, tag="ccol")
        bor_sb = persist.tile([P, DIM], F32, tag="bor")
        ident = persist.tile([P, P], BF16, tag="ident")

        nc.scalar.dma_start(out=ccol_sb[:], in_=ccol[:])
        nc.scalar.dma_start(out=bor_sb[:], in_=bor[:])
        make_identity(nc, ident[:])

        # tiny warmup AllToAll: absorbs cross-core launch skew during phase A
        # so the first real exchange starts promptly.
        wdram = ctx.enter_context(tc.tile_pool(name="wdram", bufs=1, space="DRAM"))
        warm_in = wdram.tile([NCORES, 16], BF16, tag="warm_in", name="warm_in")
        warm_out = wdram.tile([NCORES, 16], BF16, tag="warm_out", name="warm_out")
        nc.gpsimd.collective_compute(
            "AllToAll", AT.bypass, replica_groups=[list(range(NCORES))],
            ins=[warm_in[:].opt()], outs=[warm_out[:].opt()])

        # Q'/K' per flat token: rows 0-63 content, 64-66 ramp rows
        QT = persist.tile([67, NT], BF16, tag="QT")
        KT = persist.tile([67, NT], BF16, tag="KT")
        # eT for i-block (0,0): its QK+exp are embedded in phase A(b1),
        # where ScalarE is idle and 3 PSUM banks are free.
        eT0 = persist.tile([P, NTILE * IBLK], BF16, tag="eT0")
        # v token-major per (b, jt): [dv(192) | ones]
        vtok = persist.tile([P, B * NTILE * (DV + 1)], BF16, tag="vtok")
        # attention output feature-major (transposed), split 128/64 partitions
        attTa = persist.tile([P, NT], BF16, tag="attTa")
        attTb = persist.tile([64, NT], BF16, tag="attTb")
        gatT = persist.tile([P, DCH * NQ], BF16, tag="gatT")

        for b in range(B):
            bs = b * N
            nc.scalar.dma_start(out=KT[64:67, bs:bs + N], in_=krows[:])  # jhi; jlo; ones

        # QK group pattern per i-block: jt ranges with their lg pool kind
        GROUPS = [(0, 3, "A"), (3, 3, "B"), (6, 3, "A"), (9, 3, "B"),
                  (12, 2, "A"), (14, 2, "B")]

        # ---------------- phase A: projections ----------------
        with tc.tile_pool(name="xch", bufs=36) as xpool, \
             tc.tile_pool(name="pqk", bufs=2, space="PSUM") as qkpsum, \
             tc.tile_pool(name="pv", bufs=2, space="PSUM") as vpsum, \
             tc.tile_pool(name="ps", bufs=1, space="PSUM") as spsum, \
             tc.tile_pool(name="lg0", bufs=1, space="PSUM") as lg0pool, \
             tc.tile_pool(name="srow", bufs=2) as spool, \
             tc.tile_pool(name="sdram", bufs=2, space="DRAM") as sdram:

            def emit_qk00_group(g):
                jt0, njt, _ = GROUPS[g]
                lg = lg0pool.tile([P, 3 * IBLK], F32, tag="lg0", name=f"lg00_{g}")
                for jj in range(njt):
                    jt = jt0 + jj
                    nc.tensor.matmul(
                        lg[:, jj * IBLK:(jj + 1) * IBLK],
                        KT[:, jt * P:(jt + 1) * P],
                        QT[:, 0:IBLK], start=True, stop=True)
                nc.scalar.activation(
                    eT0[:, jt0 * IBLK:(jt0 + njt) * IBLK],
                    lg[:, 0:njt * IBLK], AF.Exp)

            xc_cache = {}
            # first x pair-block and the weight chunks load together, spread
            # across the three trigger queues, so the first matmul starts
            # as soon as its own chunk 0 operands land.
            for c in range(DCH):
                nc.sync.dma_start(out=wqk_sb[:, c * P:(c + 1) * P],
                                  in_=wqk[:, c * P:(c + 1) * P])
                xc2 = xpool.tile([P, 2 * IBLK], BF16, tag="xc", name=f"xc0_{c}")
                nc.sync.dma_start(out=xc2[:], in_=xT[c * P:(c + 1) * P, 0:2 * IBLK])
                xc_cache[c] = xc2
                nc.scalar.dma_start(
                    out=wv_sb[:, c * (DV + 1):(c + 1) * (DV + 1)],
                    in_=wv[:, c * (DV + 1):(c + 1) * (DV + 1)])
            for b in range(B):
                bs = b * N
                scol = spool.tile([P, NTILE], F32, tag="scol", name=f"scol{b}")
                for it in range(NIB):
                    bt = b * NIB + it
                    sl = slice(bt * IBLK, (bt + 1) * IBLK)
                    pqk = qkpsum.tile([P, IBLK], F32, tag="pqk", name=f"pqk{bt}")
                    for c in range(DCH):
                        if it % 2 == 0 and bt > 0:
                            xc2 = xpool.tile([P, 2 * IBLK], BF16, tag="xc", name=f"xc{bt}_{c}")
                            nc.sync.dma_start(
                                out=xc2[:], in_=xT[c * P:(c + 1) * P, bt * IBLK:(bt + 2) * IBLK])
                            xc_cache[c] = xc2
                        xr = xc_cache[c][:, (it % 2) * IBLK:(it % 2 + 1) * IBLK]
                        nc.tensor.matmul(pqk[:], wqk_sb[:, c * P:(c + 1) * P], xr,
                                         start=(c == 0), stop=(c == DCH - 1))
                    nc.vector.tensor_scalar_add(QT[0:DK, sl], pqk[0:DK, :], ccol_sb[0:DK, 0:1])
                    nc.vector.tensor_copy(KT[0:DK, sl], pqk[DK:2 * DK, :])
                    if b == 1 and 2 * it < len(GROUPS):
                        emit_qk00_group(2 * it)
                    # v token-major: 4 token-tiles of 128, x-chunk stationary;
                    # column 192 of wv is u = Wq_s @ w_h, giving s per token.
                    for tt in range(4):
                        jt = it * 4 + tt
                        pv = vpsum.tile([P, DV + 1], F32, tag="pv", name=f"pv{bt}_{tt}")
                        for c in range(DCH):
                            xrt = xc_cache[c][:, (it % 2) * IBLK + tt * P:(it % 2) * IBLK + (tt + 1) * P]
                            nc.tensor.matmul(pv[:], xrt,
                                             wv_sb[:, c * (DV + 1):(c + 1) * (DV + 1)],
                                             start=(c == 0), stop=(c == DCH - 1))
                        base = (b * NTILE + jt) * (DV + 1)
                        nc.vector.tensor_copy(vtok[:, base:base + DV], pv[:, 0:DV])
                        nc.vector.tensor_copy(scol[:, jt:jt + 1], pv[:, DV:DV + 1])
                        nc.gpsimd.memset(vtok[:, base + DV:base + DV + 1], 1.0)
                    if b == 1 and 2 * it + 1 < len(GROUPS):
                        emit_qk00_group(2 * it + 1)

                # ramp rows for batch b from the token-major s column:
                # build [8s | s | -B] as 48 columns, transpose once, bounce to QT rows
                stot = spool.tile([P, NTILE], F32, tag="stot", name=f"stot{b}")
                tmpc = spool.tile([P, NTILE], F32, tag="tmpc", name=f"tmpc{b}")
                scol3 = spool.tile([P, 3 * NTILE], BF16, tag="scol3", name=f"scol3{b}")
                nc.vector.tensor_scalar_add(stot[:], scol[:], ccol_sb[:, 1:2])
                nc.vector.tensor_scalar_mul(scol3[:, 0:NTILE], stot[:], 8.0)
                nc.vector.tensor_copy(scol3[:, NTILE:2 * NTILE], stot[:])
                nc.vector.tensor_scalar_max(tmpc[:], stot[:], 0.0)
                nc.vector.tensor_scalar(scol3[:, 2 * NTILE:3 * NTILE], tmpc[:],
                                        -float(N - 1), -CONTENT_BOUND, AT.mult, AT.add)
                psT = spsum.tile([3 * NTILE, P], BF16, tag="psT", name=f"psT{b}")
                nc.tensor.transpose(psT[:], scol3[:], ident[:])
                ssb = spool.tile([3 * NTILE, P], BF16, tag="ssb", name=f"ssb{b}")
                nc.vector.tensor_copy(ssb[:], psT[:])
                qs3 = sdram.tile([3 * NTILE, P], BF16, tag="qs3", name=f"qs3{b}")
                nc.sync.dma_start(out=qs3[:], in_=ssb[:])
                nc.sync.dma_start(
                    out=QT[64:67, bs:bs + N],
                    in_=qs3[:].rearrange("(t k) n -> t (k n)", t=3))

        # ---------- phase B+C: attention, exchanges, output projection ----------
        with tc.tile_pool(name="et", bufs=2) as epool, \
             tc.tile_pool(name="lgA", bufs=1, space="PSUM") as lgA, \
             tc.tile_pool(name="lgB", bufs=1, space="PSUM") as lgB, \
             tc.tile_pool(name="mid", bufs=2, space="PSUM") as midp, \
             tc.tile_pool(name="rz", bufs=4) as rzpool, \
             tc.tile_pool(name="an", bufs=4) as anpool, \
             tc.tile_pool(name="yo", bufs=2) as ypool, \
             tc.tile_pool(name="dram", bufs=1, space="DRAM") as dram:

            nc.sync.dma_start(out=wo_sb[:], in_=wo[:])

            anbuf = {(0, 0, "eT"): eT0}

            def emit_av(b, ib, ic):
                bs = b * N
                eT = anbuf[(b, ib, "eT")]
                av = midp.tile([P, IBLK], F32, tag="mid", name=f"av{b}_{ib}_{ic}")
                for jt in range(NTILE):
                    nc.tensor.matmul(
                        av[:, 0:DV + 1],
                        eT[:, jt * IBLK + ic * P:jt * IBLK + (ic + 1) * P],
                        vtok[:, (b * NTILE + jt) * (DV + 1):(b * NTILE + jt + 1) * (DV + 1)],
                        start=(jt == 0), stop=(jt == NTILE - 1))
                rz = rzpool.tile([P, 1], F32, tag="rz", name=f"rz{b}_{ib}_{ic}")
                nc.vector.reciprocal(rz[:], av[:, DV:DV + 1])
                an = anpool.tile([P, DV], BF16, tag="an", name=f"an{b}_{ib}_{ic}")
                nc.vector.tensor_scalar_mul(an[:], av[:, 0:DV], rz[:])
                anbuf[(b, ib, ic)] = an

            def emit_tt(b, ib, ic):
                an = anbuf.pop((b, ib, ic))
                iabs = b * N + ib * IBLK + ic * P
                tt = midp.tile([P, 2 * P], BF16, tag="mid", name=f"tt{b}_{ib}_{ic}")
                nc.tensor.transpose(tt[:, 0:P], an[:, 0:P], ident[:])
                nc.tensor.transpose(tt[0:64, P:2 * P], an[:, P:DV], ident[:])
                nc.vector.tensor_copy(attTa[:, iabs:iabs + P], tt[:, 0:P])
                nc.vector.tensor_copy(attTb[:, iabs:iabs + P], tt[0:64, P:2 * P])

            def emit_ib(b, ib, prev, extra=()):
                # QK+exp groups for (b, ib), interleaved with the av/transpose
                # chains of the previous i-block so the PE never idles on exp.
                # `extra` units (outproj ot-tiles) slot in at groups 2-4.
                bs = b * N
                isl = slice(bs + ib * IBLK, bs + (ib + 1) * IBLK)
                eT = epool.tile([P, NTILE * IBLK], BF16, tag="eT", name=f"eT{b}_{ib}")
                anbuf[(b, ib, "eT")] = eT
                units = []
                if prev is not None:
                    pb, pib = prev
                    units = [
                        lambda: emit_av(pb, pib, 0),
                        lambda: (emit_av(pb, pib, 1), emit_tt(pb, pib, 0)),
                        lambda: (emit_av(pb, pib, 2), emit_tt(pb, pib, 1)),
                        lambda: (emit_av(pb, pib, 3), emit_tt(pb, pib, 2)),
                        lambda: emit_tt(pb, pib, 3),
                    ]
                # QK groups run two slots ahead of the av/outproj units so the
                # exp stream on ScalarE never starves.
                for g, (jt0, njt, pk) in enumerate(GROUPS):
                    pool, tag = (lgA, "lga") if pk == "A" else (lgB, "lgb")
                    lg = pool.tile([P, 3 * IBLK], F32, tag=tag, name=f"lg{b}_{ib}_{g}")
                    for jj in range(njt):
                        jt = jt0 + jj
                        nc.tensor.matmul(
                            lg[:, jj * IBLK:(jj + 1) * IBLK],
                            KT[:, bs + jt * P:bs + (jt + 1) * P],
                            QT[:, isl], start=True, stop=True)
                    nc.scalar.activation(
                        eT[:, jt0 * IBLK:(jt0 + njt) * IBLK],
                        lg[:, 0:njt * IBLK], AF.Exp)
                    if g >= 2 and len(units) > g - 2:
                        units[g - 2]()
                    if g >= 3 and len(extra) > g - 3:
                        extra[g - 3]()
                if len(units) > 4:
                    units[4]()

            def flush_ib(b, ib, extra=()):
                emit_av(b, ib, 0)
                emit_av(b, ib, 1)
                emit_tt(b, ib, 0)
                if len(extra) > 0:
                    extra[0]()
                emit_av(b, ib, 2)
                emit_tt(b, ib, 1)
                if len(extra) > 1:
                    extra[1]()
                emit_av(b, ib, 3)
                emit_tt(b, ib, 2)
                emit_tt(b, ib, 3)
                if len(extra) > 2:
                    extra[2]()

            a2a_bufs = {}

            def do_exchange(b, h):
                off = b * N + h * (N // 2)
                a2a_in = dram.tile([NCORES * DV, GRAN], BF16,
                                   name=f"a2a_in{b}_{h}", tag=f"a2a_in{b}_{h}")
                a2a_out = dram.tile([NCORES * DV, GRAN], BF16,
                                    name=f"a2a_out{b}_{h}", tag=f"a2a_out{b}_{h}")
                for g in range(NCORES):
                    eng = nc.gpsimd if g % 2 == 0 else nc.sync
                    eng.dma_start(out=a2a_in[g * DV:g * DV + P, :],
                                  in_=attTa[:, off + g * GRAN:off + (g + 1) * GRAN])
                    eng.dma_start(out=a2a_in[g * DV + P:(g + 1) * DV, :],
                                  in_=attTb[:, off + g * GRAN:off + (g + 1) * GRAN])
                nc.gpsimd.collective_compute(
                    "AllToAll", AT.bypass,
                    replica_groups=[list(range(NCORES))],
                    ins=[a2a_in[:].opt()], outs=[a2a_out[:].opt()])
                a2a_bufs[(b, h)] = a2a_out

            def do_gather(b, h):
                # deferred until just before the consuming outproj tiles so no
                # earlier-emitted reader of gatT serializes behind it.
                a2a_out = a2a_bufs[(b, h)]
                rb = (b * 2 + h) * GRAN
                engs = [nc.sync, nc.scalar, nc.gpsimd]
                for c in range(DCH):
                    eng = engs[c % len(engs)]
                    eng.dma_start(out=gatT[:, c * NQ + rb:c * NQ + rb + GRAN],
                                  in_=a2a_out[c * P:(c + 1) * P, :])

            def outproj_ot(b, t, ot):
                rb = (b * 2 + t) * GRAN
                yp = midp.tile([P, IBLK], F32, tag="mid", name=f"yp{b}_{t}_{ot}")
                for kc in range(DCH):
                    nc.tensor.matmul(
                        yp[:], gatT[:, kc * NQ + rb:kc * NQ + rb + GRAN],
                        wo_sb[:, kc * DIM + ot * IBLK:kc * DIM + (ot + 1) * IBLK],
                        start=(kc == 0), stop=(kc == DCH - 1))
                yo = ypool.tile([P, IBLK], F32, tag="yo", name=f"yo{b}_{t}_{ot}")
                nc.vector.tensor_add(yo[:], yp[:], bor_sb[:, ot * IBLK:(ot + 1) * IBLK])
                nc.sync.dma_start(
                    out=out[rb:rb + GRAN, ot * IBLK:(ot + 1) * IBLK], in_=yo[:])

            def cunits(b, t):
                def first():
                    do_gather(b, t)
                    outproj_ot(b, t, 0)
                return [first] + [lambda ot=ot: outproj_ot(b, t, ot)
                                  for ot in range(1, DIM // IBLK)]

            # emission order: i-blocks software-pipelined (QK of ib with av of
            # ib-1); exchanges fire as soon as their half's attT is emitted;
            # b0's outproj ot-tiles slot into b1's attention as extra units.
            # outproj(1,0) runs after the last exchange is triggered, filling
            # the wait for the final collective.
            emit_ib(0, 1, (0, 0))
            emit_ib(0, 2, (0, 1))
            do_exchange(0, 0)
            emit_ib(0, 3, (0, 2))
            emit_ib(1, 0, (0, 3))
            do_exchange(0, 1)
            emit_ib(1, 1, (1, 0), extra=cunits(0, 0))
            emit_ib(1, 2, (1, 1), extra=cunits(0, 1))
            do_exchange(1, 0)
            emit_ib(1, 3, (1, 2))
            flush_ib(1, 3)
            do_exchange(1, 1)
            for u in cunits(1, 0):
                u()
            for u in cunits(1, 1):
                u()


def build_nc():
    nc = bacc_mod.Bacc(None, target_bir_lowering=False, debug=False)
    xT = nc.declare_dram_parameter("xT", [DIM, NT], BF16, isOutput=False)
    wqk = nc.declare_dram_parameter("wqk", [P, DCH * P], BF16, isOutput=False)
    wv = nc.declare_dram_parameter("wv", [P, DCH * (DV + 1)], BF16, isOutput=False)
    wo = nc.declare_dram_parameter("wo", [P, DCH * DIM], BF16, isOutput=False)
    krows = nc.declare_dram_parameter("krows", [3, N], BF16, isOutput=False)
    ccol = nc.declare_dram_parameter("ccol", [P, 2], F32, isOutput=False)
    bor = nc.declare_dram_parameter("bor", [P, DIM], F32, isOutput=False)
    out = nc.declare_dram_parameter("out", [NQ, DIM], F32, isOutput=True)
    with tile.TileContext(nc) as tc:
        _build_body(nc, tc, xT, wqk, wv, wo, krows, ccol, bor, out)
    nc.compile()
    return nc


def _in_maps(x, Wq, Wk, Wv, W_rel, Wo, bo, rcb, rpb):
    scale = np.float32(DK ** -0.5)
    Wq_s = (Wq * scale).astype(np.float32)
    iota = np.arange(N, dtype=np.float32)
    jhi = np.floor(iota / 8)
    jlo = iota - 8 * jhi
    krows = np.stack([jhi, jlo, np.ones(N, np.float32)]).astype(ml_dtypes.bfloat16)
    bor = np.broadcast_to(bo.astype(np.float32), (P, DIM)).copy()
    xTb = np.ascontiguousarray(np.concatenate([x[0].T, x[1].T], axis=1)).astype(ml_dtypes.bfloat16)

    def chunk(w):  # [DIM, M] -> [P, DCH*M] (d-chunk-major)
        m = w.shape[1]
        return np.ascontiguousarray(
            w.reshape(DCH, P, m).transpose(1, 0, 2).reshape(P, DCH * m)).astype(ml_dtypes.bfloat16)

    wo_ch = chunk(Wo)

    maps = []
    for h in range(NCORES):
        qs, ks = Wq_s[:, h * DK:(h + 1) * DK], Wk[:, h * DK:(h + 1) * DK]
        vs = Wv[:, h * DV:(h + 1) * DV]
        w_h = W_rel[0, h * DK:(h + 1) * DK]
        u = (qs @ w_h)[:, None]          # s per token = x @ u (+ rpb@w_h)
        corr = np.float32(rpb[h] @ w_h)
        ccol = np.zeros((P, 2), np.float32)
        ccol[0:DK, 0] = rcb[h]
        ccol[:, 1] = corr
        maps.append({
            "xT": xTb,
            "wqk": chunk(np.concatenate([qs, ks], axis=1)),
            "wv": chunk(np.concatenate([vs, u], axis=1)),
            "wo": wo_ch,
            "krows": krows,
            "ccol": ccol,
            "bor": bor,
        })
    return maps


def kernel(x, Wq, Wk, Wv, W_rel, Wo, bo, rel_content_bias, rel_pos_bias):
    x = np.asarray(x, np.float32)
    rcb = np.asarray(rel_content_bias, np.float32)[0, :, 0, :]
    rpb = np.asarray(rel_pos_bias, np.float32)[0, :, 0, :]
    if "nc" not in _CACHE:
        _CACHE["nc"] = build_nc()
    nc = _CACHE["nc"]
    maps = _in_maps(x, np.asarray(Wq, np.float32), np.asarray(Wk, np.float32),
                    np.asarray(Wv, np.float32), np.asarray(W_rel, np.float32),
                    np.asarray(Wo, np.float32), np.asarray(bo, np.float32), rcb, rpb)
    # first execution pays NEFF-load / launch-skew costs on all 8 cores;
    # run once to warm, then execute for real.
    run_bass_kernel_spmd(nc, maps, core_ids=list(range(NCORES)))
    res = run_bass_kernel_spmd(nc, maps, core_ids=list(range(NCORES)))
    out = np.zeros((B, N, DIM), np.float32)
    for c in range(NCORES):
        o = res.results[c]["out"]
        for b in range(B):
            for h in range(2):
                out[b, h * 1024 + c * GRAN:h * 1024 + (c + 1) * GRAN, :] = \
                    o[(b * 2 + h) * GRAN:(b * 2 + h + 1) * GRAN]
    return out.reshape(B, N, DIM)


# revision 49
# speedup vs baseline: 1.0948x; 1.0948x over previous
"""Distributed Trainium2 kernel: relative-position multi-head attention.

B=2, N=2048, DIM=1536, H=8, DK=64, DV=192.

Sharding: one head per core, both batches (8 heads / 8 cores).  Each core
projects q/k/v for its head over all 4096 tokens, runs attention, transposes
its attention output to feature-major, then four 8-core AllToAlls (one per
(batch, half)) exchange 128-row granules: core c ends up with all 8 heads'
outputs for rows [b*2048 + h*1024 + c*128, +128), and computes those rows of
the output projection.

Math: rel_k = distances @ W_rel is rank-1, so after relative_shift the
positional logits are s_i*(j-i) with s_i = (q_i*scale+rpb)@w_h.  The -s_i*i
term is constant per softmax row and drops under softmax.  So
logits = (q*scale+rcb)@k^T + s_i*j, realized as 3 extra contraction rows of
the QK^T matmul: [jhi, jlo, ones] on the K side and [8s, s, -B] on the Q
side (j = 8*jhi + jlo keeps the ramp exact in bf16), where B upper-bounds
the row max so exp cannot overflow; it cancels exactly in softmax.
s = w_h^T @ (q*scale + rcb) + (rpb-rcb)@w_h is computed by one extra matmul
against the already-projected QT content rows.
"""

import contextlib
import os
import sys

if os.path.isdir("/opt/trn_rl_repo") and "/opt/trn_rl_repo" not in sys.path:
    sys.path.insert(0, "/opt/trn_rl_repo")

import ml_dtypes
import numpy as np

import concourse.bass as bass
import concourse.bacc as bacc_mod
import concourse.mybir as mybir
import concourse.tile as tile
from concourse.bass_utils import run_bass_kernel_spmd
from concourse.masks import make_identity

B, N, DIM, H, DK, DV = 2, 2048, 1536, 8, 64, 192
NCORES = 8
NT = B * N               # 4096 flat tokens
NQ = NT // NCORES        # 512 output rows per core
P = 128
DCH = DIM // P           # 12 projection contraction chunks
NTILE = N // P           # 16 token tiles per batch
IBLK = 512
NIB = N // IBLK          # 4 i-blocks per batch
GRAN = 128               # output-row granule per core per (batch, half)
F32 = mybir.dt.float32
BF16 = mybir.dt.bfloat16
AT = mybir.AluOpType
AF = mybir.ActivationFunctionType
CONTENT_BOUND = 48.0

_CACHE = {}


def _build_body(nc, tc, xT, wqk, wv, wo, krows, ccol, bor, out):
    ctx = contextlib.ExitStack()
    with ctx:
        persist = ctx.enter_context(tc.tile_pool(name="persist", bufs=1))

        wqk_sb = persist.tile([P, DCH * P], BF16, tag="wqk")
        wv_sb = persist.tile([P, DCH * (DV + 1)], BF16, tag="wv")
        wo_sb = persist.tile([P, DCH * DIM], BF16, tag="wo")
        ccol_sb = persist.tile([P, 2], F32, tag="ccol")
        bor_sb = persist.tile([P, DIM], F32, tag="bor")
        ident = persist.tile([P, P], BF16, tag="ident")

        nc.scalar.dma_start(out=ccol_sb[:], in_=ccol[:])
        nc.scalar.dma_start(out=bor_sb[:], in_=bor[:])
        make_identity(nc, ident[:])

        # tiny warmup AllToAll: absorbs cross-core launch skew during phase A
        # so the first real exchange starts promptly.
        wdram = ctx.enter_context(tc.tile_pool(name="wdram", bufs=1, space="DRAM"))
        warm_in = wdram.tile([NCORES, 16], BF16, tag="warm_in", name="warm_in")
        warm_out = wdram.tile([NCORES, 16], BF16, tag="warm_out", name="warm_out")
        nc.gpsimd.collective_compute(
            "AllToAll", AT.bypass, replica_groups=[list(range(NCORES))],
            ins=[warm_in[:].opt()], outs=[warm_out[:].opt()])

        # Q'/K' per flat token: rows 0-63 content, 64-66 ramp rows
        QT = persist.tile([67, NT], BF16, tag="QT")
        KT = persist.tile([67, NT], BF16, tag="KT")
        # v token-major per (b, jt): [dv(192) | ones]
        vtok = persist.tile([P, B * NTILE * (DV + 1)], BF16, tag="vtok")
        # attention output feature-major (transposed), split 128/64 partitions
        attTa = persist.tile([P, NT], BF16, tag="attTa")
        attTb = persist.tile([64, NT], BF16, tag="attTb")
        gatT = persist.tile([P, DCH * NQ], BF16, tag="gatT")

        for b in range(B):
            bs = b * N
            nc.scalar.dma_start(out=KT[64:67, bs:bs + N], in_=krows[:])  # jhi; jlo; ones

        # ---------------- phase A: projections ----------------
        with tc.tile_pool(name="xch", bufs=38) as xpool, \
             tc.tile_pool(name="pqk", bufs=2, space="PSUM") as qkpsum, \
             tc.tile_pool(name="pv", bufs=2, space="PSUM") as vpsum, \
             tc.tile_pool(name="ps", bufs=1, space="PSUM") as spsum, \
             tc.tile_pool(name="srow", bufs=2) as spool, \
             tc.tile_pool(name="sdram", bufs=2, space="DRAM") as sdram:

            xc_cache = {}
            # first x pair-block and the weight chunks load together, spread
            # across the three trigger queues, so the first matmul starts
            # as soon as its own chunk 0 operands land.
            for c in range(DCH):
                nc.sync.dma_start(out=wqk_sb[:, c * P:(c + 1) * P],
                                  in_=wqk[:, c * P:(c + 1) * P])
                xc2 = xpool.tile([P, 2 * IBLK], BF16, tag="xc", name=f"xc0_{c}")
                nc.sync.dma_start(out=xc2[:], in_=xT[c * P:(c + 1) * P, 0:2 * IBLK])
                xc_cache[c] = xc2
                nc.scalar.dma_start(
                    out=wv_sb[:, c * (DV + 1):(c + 1) * (DV + 1)],
                    in_=wv[:, c * (DV + 1):(c + 1) * (DV + 1)])
            for b in range(B):
                bs = b * N
                scol = spool.tile([P, NTILE], F32, tag="scol", name=f"scol{b}")
                for it in range(NIB):
                    bt = b * NIB + it
                    sl = slice(bt * IBLK, (bt + 1) * IBLK)
                    pqk = qkpsum.tile([P, IBLK], F32, tag="pqk", name=f"pqk{bt}")
                    for c in range(DCH):
                        if it % 2 == 0 and bt > 0:
                            xc2 = xpool.tile([P, 2 * IBLK], BF16, tag="xc", name=f"xc{bt}_{c}")
                            nc.sync.dma_start(
                                out=xc2[:], in_=xT[c * P:(c + 1) * P, bt * IBLK:(bt + 2) * IBLK])
                            xc_cache[c] = xc2
                        xr = xc_cache[c][:, (it % 2) * IBLK:(it % 2 + 1) * IBLK]
                        nc.tensor.matmul(pqk[:], wqk_sb[:, c * P:(c + 1) * P], xr,
                                         start=(c == 0), stop=(c == DCH - 1))
                    nc.vector.tensor_scalar_add(QT[0:DK, sl], pqk[0:DK, :], ccol_sb[0:DK, 0:1])
                    nc.vector.tensor_copy(KT[0:DK, sl], pqk[DK:2 * DK, :])
                    # v token-major: 4 token-tiles of 128, x-chunk stationary;
                    # column 192 of wv is u = Wq_s @ w_h, giving s per token.
                    for tt in range(4):
                        jt = it * 4 + tt
                        pv = vpsum.tile([P, DV + 1], F32, tag="pv", name=f"pv{bt}_{tt}")
                        for c in range(DCH):
                            xrt = xc_cache[c][:, (it % 2) * IBLK + tt * P:(it % 2) * IBLK + (tt + 1) * P]
                            nc.tensor.matmul(pv[:], xrt,
                                             wv_sb[:, c * (DV + 1):(c + 1) * (DV + 1)],
                                             start=(c == 0), stop=(c == DCH - 1))
                        base = (b * NTILE + jt) * (DV + 1)
                        nc.vector.tensor_copy(vtok[:, base:base + DV], pv[:, 0:DV])
                        nc.vector.tensor_copy(scol[:, jt:jt + 1], pv[:, DV:DV + 1])
                        nc.gpsimd.memset(vtok[:, base + DV:base + DV + 1], 1.0)

                # ramp rows for batch b from the token-major s column:
                # build [8s | s | -B] as 48 columns, transpose once, bounce to QT rows
                stot = spool.tile([P, NTILE], F32, tag="stot", name=f"stot{b}")
                tmpc = spool.tile([P, NTILE], F32, tag="tmpc", name=f"tmpc{b}")
                scol3 = spool.tile([P, 3 * NTILE], BF16, tag="scol3", name=f"scol3{b}")
                nc.vector.tensor_scalar_add(stot[:], scol[:], ccol_sb[:, 1:2])
                nc.vector.tensor_scalar_mul(scol3[:, 0:NTILE], stot[:], 8.0)
                nc.vector.tensor_copy(scol3[:, NTILE:2 * NTILE], stot[:])
                nc.vector.tensor_scalar_max(tmpc[:], stot[:], 0.0)
                nc.vector.tensor_scalar(scol3[:, 2 * NTILE:3 * NTILE], tmpc[:],
                                        -float(N - 1), -CONTENT_BOUND, AT.mult, AT.add)
                psT = spsum.tile([3 * NTILE, P], BF16, tag="psT", name=f"psT{b}")
                nc.tensor.transpose(psT[:], scol3[:], ident[:])
                ssb = spool.tile([3 * NTILE, P], BF16, tag="ssb", name=f"ssb{b}")
                nc.vector.tensor_copy(ssb[:], psT[:])
                qs3 = sdram.tile([3 * NTILE, P], BF16, tag="qs3", name=f"qs3{b}")
                nc.sync.dma_start(out=qs3[:], in_=ssb[:])
                nc.sync.dma_start(
                    out=QT[64:67, bs:bs + N],
                    in_=qs3[:].rearrange("(t k) n -> t (k n)", t=3))

        # ---------- phase B+C: attention, exchanges, output projection ----------
        with tc.tile_pool(name="et", bufs=2) as epool, \
             tc.tile_pool(name="lgA", bufs=1, space="PSUM") as lgA, \
             tc.tile_pool(name="lgB", bufs=1, space="PSUM") as lgB, \
             tc.tile_pool(name="mid", bufs=2, space="PSUM") as midp, \
             tc.tile_pool(name="rz", bufs=4) as rzpool, \
             tc.tile_pool(name="an", bufs=4) as anpool, \
             tc.tile_pool(name="yo", bufs=2) as ypool, \
             tc.tile_pool(name="dram", bufs=1, space="DRAM") as dram:

            nc.sync.dma_start(out=wo_sb[:], in_=wo[:])

            # QK group pattern per i-block: jt ranges with their lg pool
            GROUPS = [(0, 3, "A"), (3, 3, "B"), (6, 3, "A"), (9, 3, "B"),
                      (12, 2, "A"), (14, 2, "B")]
            anbuf = {}

            def emit_av(b, ib, ic):
                bs = b * N
                eT = anbuf[(b, ib, "eT")]
                av = midp.tile([P, IBLK], F32, tag="mid", name=f"av{b}_{ib}_{ic}")
                for jt in range(NTILE):
                    nc.tensor.matmul(
                        av[:, 0:DV + 1],
                        eT[:, jt * IBLK + ic * P:jt * IBLK + (ic + 1) * P],
                        vtok[:, (b * NTILE + jt) * (DV + 1):(b * NTILE + jt + 1) * (DV + 1)],
                        start=(jt == 0), stop=(jt == NTILE - 1))
                rz = rzpool.tile([P, 1], F32, tag="rz", name=f"rz{b}_{ib}_{ic}")
                nc.vector.reciprocal(rz[:], av[:, DV:DV + 1])
                an = anpool.tile([P, DV], BF16, tag="an", name=f"an{b}_{ib}_{ic}")
                nc.vector.tensor_scalar_mul(an[:], av[:, 0:DV], rz[:])
                anbuf[(b, ib, ic)] = an

            def emit_tt(b, ib, ic):
                an = anbuf.pop((b, ib, ic))
                iabs = b * N + ib * IBLK + ic * P
                tt = midp.tile([P, 2 * P], BF16, tag="mid", name=f"tt{b}_{ib}_{ic}")
                nc.tensor.transpose(tt[:, 0:P], an[:, 0:P], ident[:])
                nc.tensor.transpose(tt[0:64, P:2 * P], an[:, P:DV], ident[:])
                nc.vector.tensor_copy(attTa[:, iabs:iabs + P], tt[:, 0:P])
                nc.vector.tensor_copy(attTb[:, iabs:iabs + P], tt[0:64, P:2 * P])

            def emit_ib(b, ib, prev, extra=()):
                # QK+exp groups for (b, ib), interleaved with the av/transpose
                # chains of the previous i-block so the PE never idles on exp.
                # `extra` units (outproj ot-tiles) slot in at groups 2-4.
                bs = b * N
                isl = slice(bs + ib * IBLK, bs + (ib + 1) * IBLK)
                eT = epool.tile([P, NTILE * IBLK], BF16, tag="eT", name=f"eT{b}_{ib}")
                anbuf[(b, ib, "eT")] = eT
                units = []
                if prev is not None:
                    pb, pib = prev
                    units = [
                        lambda: emit_av(pb, pib, 0),
                        lambda: (emit_av(pb, pib, 1), emit_tt(pb, pib, 0)),
                        lambda: (emit_av(pb, pib, 2), emit_tt(pb, pib, 1)),
                        lambda: (emit_av(pb, pib, 3), emit_tt(pb, pib, 2)),
                        lambda: emit_tt(pb, pib, 3),
                    ]
                # QK groups run two slots ahead of the av/outproj units so the
                # exp stream on ScalarE never starves.
                for g, (jt0, njt, pk) in enumerate(GROUPS):
                    pool, tag = (lgA, "lga") if pk == "A" else (lgB, "lgb")
                    lg = pool.tile([P, 3 * IBLK], F32, tag=tag, name=f"lg{b}_{ib}_{g}")
                    for jj in range(njt):
                        jt = jt0 + jj
                        nc.tensor.matmul(
                            lg[:, jj * IBLK:(jj + 1) * IBLK],
                            KT[:, bs + jt * P:bs + (jt + 1) * P],
                            QT[:, isl], start=True, stop=True)
                    nc.scalar.activation(
                        eT[:, jt0 * IBLK:(jt0 + njt) * IBLK],
                        lg[:, 0:njt * IBLK], AF.Exp)
                    if g >= 2 and len(units) > g - 2:
                        units[g - 2]()
                    if g >= 3 and len(extra) > g - 3:
                        extra[g - 3]()
                if len(units) > 4:
                    units[4]()

            def flush_ib(b, ib, extra=()):
                emit_av(b, ib, 0)
                emit_av(b, ib, 1)
                emit_tt(b, ib, 0)
                if len(extra) > 0:
                    extra[0]()
                emit_av(b, ib, 2)
                emit_tt(b, ib, 1)
                if len(extra) > 1:
                    extra[1]()
                emit_av(b, ib, 3)
                emit_tt(b, ib, 2)
                emit_tt(b, ib, 3)
                if len(extra) > 2:
                    extra[2]()

            a2a_bufs = {}

            def do_exchange(b, h):
                off = b * N + h * (N // 2)
                a2a_in = dram.tile([NCORES * DV, GRAN], BF16,
                                   name=f"a2a_in{b}_{h}", tag=f"a2a_in{b}_{h}")
                a2a_out = dram.tile([NCORES * DV, GRAN], BF16,
                                    name=f"a2a_out{b}_{h}", tag=f"a2a_out{b}_{h}")
                for g in range(NCORES):
                    eng = nc.gpsimd if g % 2 == 0 else nc.sync
                    eng.dma_start(out=a2a_in[g * DV:g * DV + P, :],
                                  in_=attTa[:, off + g * GRAN:off + (g + 1) * GRAN])
                    eng.dma_start(out=a2a_in[g * DV + P:(g + 1) * DV, :],
                                  in_=attTb[:, off + g * GRAN:off + (g + 1) * GRAN])
                nc.gpsimd.collective_compute(
                    "AllToAll", AT.bypass,
                    replica_groups=[list(range(NCORES))],
                    ins=[a2a_in[:].opt()], outs=[a2a_out[:].opt()])
                a2a_bufs[(b, h)] = a2a_out

            def do_gather(b, h):
                # deferred until just before the consuming outproj tiles so no
                # earlier-emitted reader of gatT serializes behind it.
                a2a_out = a2a_bufs[(b, h)]
                rb = (b * 2 + h) * GRAN
                engs = [nc.sync, nc.scalar, nc.gpsimd]
                for c in range(DCH):
                    eng = engs[c % len(engs)]
                    eng.dma_start(out=gatT[:, c * NQ + rb:c * NQ + rb + GRAN],
                                  in_=a2a_out[c * P:(c + 1) * P, :])

            def outproj_ot(b, t, ot):
                rb = (b * 2 + t) * GRAN
                yp = midp.tile([P, IBLK], F32, tag="mid", name=f"yp{b}_{t}_{ot}")
                for kc in range(DCH):
                    nc.tensor.matmul(
                        yp[:], gatT[:, kc * NQ + rb:kc * NQ + rb + GRAN],
                        wo_sb[:, kc * DIM + ot * IBLK:kc * DIM + (ot + 1) * IBLK],
                        start=(kc == 0), stop=(kc == DCH - 1))
                yo = ypool.tile([P, IBLK], F32, tag="yo", name=f"yo{b}_{t}_{ot}")
                nc.vector.tensor_add(yo[:], yp[:], bor_sb[:, ot * IBLK:(ot + 1) * IBLK])
                nc.sync.dma_start(
                    out=out[rb:rb + GRAN, ot * IBLK:(ot + 1) * IBLK], in_=yo[:])

            def cunits(b, t):
                def first():
                    do_gather(b, t)
                    outproj_ot(b, t, 0)
                return [first] + [lambda ot=ot: outproj_ot(b, t, ot)
                                  for ot in range(1, DIM // IBLK)]

            # emission order: i-blocks software-pipelined (QK of ib with av of
            # ib-1); exchanges fire as soon as their half's attT is emitted;
            # b0's outproj ot-tiles slot into b1's attention as extra units.
            # outproj(1,0) runs after the last exchange is triggered, filling
            # the wait for the final collective.
            emit_ib(0, 0, None)
            emit_ib(0, 1, (0, 0))
            emit_ib(0, 2, (0, 1))
            do_exchange(0, 0)
            emit_ib(0, 3, (0, 2))
            emit_ib(1, 0, (0, 3))
            do_exchange(0, 1)
            emit_ib(1, 1, (1, 0), extra=cunits(0, 0))
            emit_ib(1, 2, (1, 1), extra=cunits(0, 1))
            do_exchange(1, 0)
            emit_ib(1, 3, (1, 2))
            flush_ib(1, 3)
            do_exchange(1, 1)
            for u in cunits(1, 0):
                u()
            for u in cunits(1, 1):
                u()


def build_nc():
    nc = bacc_mod.Bacc(None, target_bir_lowering=False, debug=False)
    xT = nc.declare_dram_parameter("xT", [DIM, NT], BF16, isOutput=False)
    wqk = nc.declare_dram_parameter("wqk", [P, DCH * P], BF16, isOutput=False)
    wv = nc.declare_dram_parameter("wv", [P, DCH * (DV + 1)], BF16, isOutput=False)
    wo = nc.declare_dram_parameter("wo", [P, DCH * DIM], BF16, isOutput=False)
    krows = nc.declare_dram_parameter("krows", [3, N], BF16, isOutput=False)
    ccol = nc.declare_dram_parameter("ccol", [P, 2], F32, isOutput=False)
    bor = nc.declare_dram_parameter("bor", [P, DIM], F32, isOutput=False)
    out = nc.declare_dram_parameter("out", [NQ, DIM], F32, isOutput=True)
    with tile.TileContext(nc) as tc:
        _build_body(nc, tc, xT, wqk, wv, wo, krows, ccol, bor, out)
    nc.compile()
    return nc


def _in_maps(x, Wq, Wk, Wv, W_rel, Wo, bo, rcb, rpb):
    scale = np.float32(DK ** -0.5)
    Wq_s = (Wq * scale).astype(np.float32)
    iota = np.arange(N, dtype=np.float32)
    jhi = np.floor(iota / 8)
    jlo = iota - 8 * jhi
    krows = np.stack([jhi, jlo, np.ones(N, np.float32)]).astype(ml_dtypes.bfloat16)
    bor = np.broadcast_to(bo.astype(np.float32), (P, DIM)).copy()
    xTb = np.ascontiguousarray(np.concatenate([x[0].T, x[1].T], axis=1)).astype(ml_dtypes.bfloat16)

    def chunk(w):  # [DIM, M] -> [P, DCH*M] (d-chunk-major)
        m = w.shape[1]
        return np.ascontiguousarray(
            w.reshape(DCH, P, m).transpose(1, 0, 2).reshape(P, DCH * m)).astype(ml_dtypes.bfloat16)

    wo_ch = chunk(Wo)

    maps = []
    for h in range(NCORES):
        qs, ks = Wq_s[:, h * DK:(h + 1) * DK], Wk[:, h * DK:(h + 1) * DK]
        vs = Wv[:, h * DV:(h + 1) * DV]
        w_h = W_rel[0, h * DK:(h + 1) * DK]
        u = (qs @ w_h)[:, None]          # s per token = x @ u (+ rpb@w_h)
        corr = np.float32(rpb[h] @ w_h)
        ccol = np.zeros((P, 2), np.float32)
        ccol[0:DK, 0] = rcb[h]
        ccol[:, 1] = corr
        maps.append({
            "xT": xTb,
            "wqk": chunk(np.concatenate([qs, ks], axis=1)),
            "wv": chunk(np.concatenate([vs, u], axis=1)),
            "wo": wo_ch,
            "krows": krows,
            "ccol": ccol,
            "bor": bor,
        })
    return maps


def kernel(x, Wq, Wk, Wv, W_rel, Wo, bo, rel_content_bias, rel_pos_bias):
    x = np.asarray(x, np.float32)
    rcb = np.asarray(rel_content_bias, np.float32)[0, :, 0, :]
    rpb = np.asarray(rel_pos_bias, np.float32)[0, :, 0, :]
    if "nc" not in _CACHE:
        _CACHE["nc"] = build_nc()
    nc = _CACHE["nc"]
    maps = _in_maps(x, np.asarray(Wq, np.float32), np.asarray(Wk, np.float32),
                    np.asarray(Wv, np.float32), np.asarray(W_rel, np.float32),
                    np.asarray(Wo, np.float32), np.asarray(bo, np.float32), rcb, rpb)
    # first execution pays NEFF-load / launch-skew costs on all 8 cores;
    # run once to warm, then execute for real.
    run_bass_kernel_spmd(nc, maps, core_ids=list(range(NCORES)))
    res = run_bass_kernel_spmd(nc, maps, core_ids=list(range(NCORES)))
    out = np.zeros((B, N, DIM), np.float32)
    for c in range(NCORES):
        o = res.results[c]["out"]
        for b in range(B):
            for h in range(2):
                out[b, h * 1024 + c * GRAN:h * 1024 + (c + 1) * GRAN, :] = \
                    o[(b * 2 + h) * GRAN:(b * 2 + h + 1) * GRAN]
    return out.reshape(B, N, DIM)


# revision 50
# speedup vs baseline: 1.2289x; 1.1225x over previous
"""Distributed Trainium2 kernel: relative-position multi-head attention.

B=2, N=2048, DIM=1536, H=8, DK=64, DV=192.

Sharding: one head per core, both batches (8 heads / 8 cores).  Each core
projects q/k/v for its head over all 4096 tokens, runs attention, transposes
its attention output to feature-major, then four 8-core AllToAlls (one per
(batch, half)) exchange 128-row granules: core c ends up with all 8 heads'
outputs for rows [b*2048 + h*1024 + c*128, +128), and computes those rows of
the output projection.

Math: rel_k = distances @ W_rel is rank-1, so after relative_shift the
positional logits are s_i*(j-i) with s_i = (q_i*scale+rpb)@w_h.  The -s_i*i
term is constant per softmax row and drops under softmax.  So
logits = (q*scale+rcb)@k^T + s_i*j, realized as 3 extra contraction rows of
the QK^T matmul: [jhi, jlo, ones] on the K side and [8s, s, -B] on the Q
side (j = 8*jhi + jlo keeps the ramp exact in bf16), where B upper-bounds
the row max so exp cannot overflow; it cancels exactly in softmax.
s = w_h^T @ (q*scale + rcb) + (rpb-rcb)@w_h is computed by one extra matmul
against the already-projected QT content rows.
"""

import contextlib
import os
import sys

if os.path.isdir("/opt/trn_rl_repo") and "/opt/trn_rl_repo" not in sys.path:
    sys.path.insert(0, "/opt/trn_rl_repo")

import ml_dtypes
import numpy as np

import concourse.bass as bass
import concourse.bacc as bacc_mod
import concourse.mybir as mybir
import concourse.tile as tile
from concourse.bass_utils import run_bass_kernel_spmd
from concourse.masks import make_identity

B, N, DIM, H, DK, DV = 2, 2048, 1536, 8, 64, 192
NCORES = 8
NT = B * N               # 4096 flat tokens
NQ = NT // NCORES        # 512 output rows per core
P = 128
DCH = DIM // P           # 12 projection contraction chunks
NTILE = N // P           # 16 token tiles per batch
IBLK = 512
NIB = N // IBLK          # 4 i-blocks per batch
GRAN = 128               # output-row granule per core per (batch, half)
F32 = mybir.dt.float32
BF16 = mybir.dt.bfloat16
AT = mybir.AluOpType
AF = mybir.ActivationFunctionType
CONTENT_BOUND = 48.0

_CACHE = {}


def _build_body(nc, tc, xT, wqk, wv, wo, krows, ccol, bor, out):
    ctx = contextlib.ExitStack()
    with ctx:
        persist = ctx.enter_context(tc.tile_pool(name="persist", bufs=1))

        wqk_sb = persist.tile([P, DCH * P], BF16, tag="wqk")
        wv_sb = persist.tile([P, DCH * (DV + 1)], BF16, tag="wv")
        wo_sb = persist.tile([P, DCH * DIM], BF16, tag="wo")
        ccol_sb = persist.tile([P, 2], F32, tag="ccol")
        bor_sb = persist.tile([P, DIM], F32, tag="bor")
        ident = persist.tile([P, P], BF16, tag="ident")

        nc.scalar.dma_start(out=ccol_sb[:], in_=ccol[:])
        nc.scalar.dma_start(out=bor_sb[:], in_=bor[:])
        make_identity(nc, ident[:])

        # tiny warmup AllToAll: absorbs cross-core launch skew during phase A
        # so the first real exchange starts promptly.
        wdram = ctx.enter_context(tc.tile_pool(name="wdram", bufs=1, space="DRAM"))
        warm_in = wdram.tile([NCORES, 16], BF16, tag="warm_in", name="warm_in")
        warm_out = wdram.tile([NCORES, 16], BF16, tag="warm_out", name="warm_out")
        nc.gpsimd.collective_compute(
            "AllToAll", AT.bypass, replica_groups=[list(range(NCORES))],
            ins=[warm_in[:].opt()], outs=[warm_out[:].opt()])

        # Q'/K' per flat token: rows 0-63 content, 64-66 ramp rows
        QT = persist.tile([67, NT], BF16, tag="QT")
        KT = persist.tile([67, NT], BF16, tag="KT")
        # v token-major per (b, jt): [dv(192) | ones]
        vtok = persist.tile([P, B * NTILE * (DV + 1)], BF16, tag="vtok")
        # attention output feature-major (transposed), split 128/64 partitions
        attTa = persist.tile([P, NT], BF16, tag="attTa")
        attTb = persist.tile([64, NT], BF16, tag="attTb")
        gatT = persist.tile([P, DCH * NQ], BF16, tag="gatT")

        for b in range(B):
            bs = b * N
            nc.scalar.dma_start(out=KT[64:67, bs:bs + N], in_=krows[:])  # jhi; jlo; ones

        # ---------------- phase A: projections ----------------
        with tc.tile_pool(name="xch", bufs=38) as xpool, \
             tc.tile_pool(name="pqk", bufs=2, space="PSUM") as qkpsum, \
             tc.tile_pool(name="pv", bufs=2, space="PSUM") as vpsum, \
             tc.tile_pool(name="ps", bufs=1, space="PSUM") as spsum, \
             tc.tile_pool(name="srow", bufs=2) as spool, \
             tc.tile_pool(name="sdram", bufs=2, space="DRAM") as sdram:

            xc_cache = {}
            # first x pair-block and the weight chunks load together, spread
            # across the three trigger queues, so the first matmul starts
            # as soon as its own chunk 0 operands land.
            for c in range(DCH):
                nc.sync.dma_start(out=wqk_sb[:, c * P:(c + 1) * P],
                                  in_=wqk[:, c * P:(c + 1) * P])
                xc2 = xpool.tile([P, 2 * IBLK], BF16, tag="xc", name=f"xc0_{c}")
                nc.sync.dma_start(out=xc2[:], in_=xT[c * P:(c + 1) * P, 0:2 * IBLK])
                xc_cache[c] = xc2
                nc.scalar.dma_start(
                    out=wv_sb[:, c * (DV + 1):(c + 1) * (DV + 1)],
                    in_=wv[:, c * (DV + 1):(c + 1) * (DV + 1)])
            for b in range(B):
                bs = b * N
                scol = spool.tile([P, NTILE], F32, tag="scol", name=f"scol{b}")
                for it in range(NIB):
                    bt = b * NIB + it
                    sl = slice(bt * IBLK, (bt + 1) * IBLK)
                    pqk = qkpsum.tile([P, IBLK], F32, tag="pqk", name=f"pqk{bt}")
                    for c in range(DCH):
                        if it % 2 == 0 and bt > 0:
                            xc2 = xpool.tile([P, 2 * IBLK], BF16, tag="xc", name=f"xc{bt}_{c}")
                            nc.sync.dma_start(
                                out=xc2[:], in_=xT[c * P:(c + 1) * P, bt * IBLK:(bt + 2) * IBLK])
                            xc_cache[c] = xc2
                        xr = xc_cache[c][:, (it % 2) * IBLK:(it % 2 + 1) * IBLK]
                        nc.tensor.matmul(pqk[:], wqk_sb[:, c * P:(c + 1) * P], xr,
                                         start=(c == 0), stop=(c == DCH - 1))
                    nc.vector.tensor_scalar_add(QT[0:DK, sl], pqk[0:DK, :], ccol_sb[0:DK, 0:1])
                    nc.vector.tensor_copy(KT[0:DK, sl], pqk[DK:2 * DK, :])
                    # v token-major: 4 token-tiles of 128, x-chunk stationary;
                    # column 192 of wv is u = Wq_s @ w_h, giving s per token.
                    for tt in range(4):
                        jt = it * 4 + tt
                        pv = vpsum.tile([P, DV + 1], F32, tag="pv", name=f"pv{bt}_{tt}")
                        for c in range(DCH):
                            xrt = xc_cache[c][:, (it % 2) * IBLK + tt * P:(it % 2) * IBLK + (tt + 1) * P]
                            nc.tensor.matmul(pv[:], xrt,
                                             wv_sb[:, c * (DV + 1):(c + 1) * (DV + 1)],
                                             start=(c == 0), stop=(c == DCH - 1))
                        base = (b * NTILE + jt) * (DV + 1)
                        nc.vector.tensor_copy(vtok[:, base:base + DV], pv[:, 0:DV])
                        nc.vector.tensor_copy(scol[:, jt:jt + 1], pv[:, DV:DV + 1])
                        nc.gpsimd.memset(vtok[:, base + DV:base + DV + 1], 1.0)

                # ramp rows for batch b from the token-major s column:
                # build [8s | s | -B] as 48 columns, transpose once, bounce to QT rows
                stot = spool.tile([P, NTILE], F32, tag="stot", name=f"stot{b}")
                tmpc = spool.tile([P, NTILE], F32, tag="tmpc", name=f"tmpc{b}")
                scol3 = spool.tile([P, 3 * NTILE], BF16, tag="scol3", name=f"scol3{b}")
                nc.vector.tensor_scalar_add(stot[:], scol[:], ccol_sb[:, 1:2])
                nc.vector.tensor_scalar_mul(scol3[:, 0:NTILE], stot[:], 8.0)
                nc.vector.tensor_copy(scol3[:, NTILE:2 * NTILE], stot[:])
                nc.vector.tensor_scalar_max(tmpc[:], stot[:], 0.0)
                nc.vector.tensor_scalar(scol3[:, 2 * NTILE:3 * NTILE], tmpc[:],
                                        -float(N - 1), -CONTENT_BOUND, AT.mult, AT.add)
                psT = spsum.tile([3 * NTILE, P], BF16, tag="psT", name=f"psT{b}")
                nc.tensor.transpose(psT[:], scol3[:], ident[:])
                ssb = spool.tile([3 * NTILE, P], BF16, tag="ssb", name=f"ssb{b}")
                nc.vector.tensor_copy(ssb[:], psT[:])
                qs3 = sdram.tile([3 * NTILE, P], BF16, tag="qs3", name=f"qs3{b}")
                nc.sync.dma_start(out=qs3[:], in_=ssb[:])
                nc.sync.dma_start(
                    out=QT[64:67, bs:bs + N],
                    in_=qs3[:].rearrange("(t k) n -> t (k n)", t=3))

        # ---------- phase B+C: attention, exchanges, output projection ----------
        with tc.tile_pool(name="et", bufs=2) as epool, \
             tc.tile_pool(name="lgA", bufs=1, space="PSUM") as lgA, \
             tc.tile_pool(name="lgB", bufs=1, space="PSUM") as lgB, \
             tc.tile_pool(name="mid", bufs=2, space="PSUM") as midp, \
             tc.tile_pool(name="rz", bufs=4) as rzpool, \
             tc.tile_pool(name="an", bufs=4) as anpool, \
             tc.tile_pool(name="yo", bufs=2) as ypool, \
             tc.tile_pool(name="dram", bufs=1, space="DRAM") as dram:

            nc.sync.dma_start(out=wo_sb[:], in_=wo[:])

            # QK group pattern per i-block: jt ranges with their lg pool
            GROUPS = [(0, 3, "A"), (3, 3, "B"), (6, 3, "A"), (9, 3, "B"),
                      (12, 2, "A"), (14, 2, "B")]
            anbuf = {}

            def emit_av(b, ib, ic):
                bs = b * N
                eT = anbuf[(b, ib, "eT")]
                av = midp.tile([P, IBLK], F32, tag="mid", name=f"av{b}_{ib}_{ic}")
                for jt in range(NTILE):
                    nc.tensor.matmul(
                        av[:, 0:DV + 1],
                        eT[:, jt * IBLK + ic * P:jt * IBLK + (ic + 1) * P],
                        vtok[:, (b * NTILE + jt) * (DV + 1):(b * NTILE + jt + 1) * (DV + 1)],
                        start=(jt == 0), stop=(jt == NTILE - 1))
                rz = rzpool.tile([P, 1], F32, tag="rz", name=f"rz{b}_{ib}_{ic}")
                nc.vector.reciprocal(rz[:], av[:, DV:DV + 1])
                an = anpool.tile([P, DV], BF16, tag="an", name=f"an{b}_{ib}_{ic}")
                nc.vector.tensor_scalar_mul(an[:], av[:, 0:DV], rz[:])
                anbuf[(b, ib, ic)] = an

            def emit_tt(b, ib, ic):
                an = anbuf.pop((b, ib, ic))
                iabs = b * N + ib * IBLK + ic * P
                tt = midp.tile([P, 2 * P], BF16, tag="mid", name=f"tt{b}_{ib}_{ic}")
                nc.tensor.transpose(tt[:, 0:P], an[:, 0:P], ident[:])
                nc.tensor.transpose(tt[0:64, P:2 * P], an[:, P:DV], ident[:])
                nc.vector.tensor_copy(attTa[:, iabs:iabs + P], tt[:, 0:P])
                nc.vector.tensor_copy(attTb[:, iabs:iabs + P], tt[0:64, P:2 * P])

            def emit_ib(b, ib, prev, extra=()):
                # QK+exp groups for (b, ib), interleaved with the av/transpose
                # chains of the previous i-block so the PE never idles on exp.
                # `extra` units (outproj ot-tiles) slot in at groups 2-4.
                bs = b * N
                isl = slice(bs + ib * IBLK, bs + (ib + 1) * IBLK)
                eT = epool.tile([P, NTILE * IBLK], BF16, tag="eT", name=f"eT{b}_{ib}")
                anbuf[(b, ib, "eT")] = eT
                units = []
                if prev is not None:
                    pb, pib = prev
                    units = [
                        lambda: emit_av(pb, pib, 0),
                        lambda: (emit_av(pb, pib, 1), emit_tt(pb, pib, 0)),
                        lambda: (emit_av(pb, pib, 2), emit_tt(pb, pib, 1)),
                        lambda: (emit_av(pb, pib, 3), emit_tt(pb, pib, 2)),
                        lambda: emit_tt(pb, pib, 3),
                    ]
                # QK groups run two slots ahead of the av/outproj units so the
                # exp stream on ScalarE never starves.
                for g, (jt0, njt, pk) in enumerate(GROUPS):
                    pool, tag = (lgA, "lga") if pk == "A" else (lgB, "lgb")
                    lg = pool.tile([P, 3 * IBLK], F32, tag=tag, name=f"lg{b}_{ib}_{g}")
                    for jj in range(njt):
                        jt = jt0 + jj
                        nc.tensor.matmul(
                            lg[:, jj * IBLK:(jj + 1) * IBLK],
                            KT[:, bs + jt * P:bs + (jt + 1) * P],
                            QT[:, isl], start=True, stop=True)
                    nc.scalar.activation(
                        eT[:, jt0 * IBLK:(jt0 + njt) * IBLK],
                        lg[:, 0:njt * IBLK], AF.Exp)
                    if g >= 2 and len(units) > g - 2:
                        units[g - 2]()
                    if g >= 3 and len(extra) > g - 3:
                        extra[g - 3]()
                if len(units) > 4:
                    units[4]()

            def flush_ib(b, ib, extra=()):
                emit_av(b, ib, 0)
                emit_av(b, ib, 1)
                emit_tt(b, ib, 0)
                if len(extra) > 0:
                    extra[0]()
                emit_av(b, ib, 2)
                emit_tt(b, ib, 1)
                if len(extra) > 1:
                    extra[1]()
                emit_av(b, ib, 3)
                emit_tt(b, ib, 2)
                emit_tt(b, ib, 3)
                if len(extra) > 2:
                    extra[2]()

            a2a_bufs = {}

            def do_exchange(b, h):
                off = b * N + h * (N // 2)
                a2a_in = dram.tile([NCORES * DV, GRAN], BF16,
                                   name=f"a2a_in{b}_{h}", tag=f"a2a_in{b}_{h}")
                a2a_out = dram.tile([NCORES * DV, GRAN], BF16,
                                    name=f"a2a_out{b}_{h}", tag=f"a2a_out{b}_{h}")
                for g in range(NCORES):
                    eng = nc.gpsimd if g % 2 == 0 else nc.sync
                    eng.dma_start(out=a2a_in[g * DV:g * DV + P, :],
                                  in_=attTa[:, off + g * GRAN:off + (g + 1) * GRAN])
                    eng.dma_start(out=a2a_in[g * DV + P:(g + 1) * DV, :],
                                  in_=attTb[:, off + g * GRAN:off + (g + 1) * GRAN])
                nc.gpsimd.collective_compute(
                    "AllToAll", AT.bypass,
                    replica_groups=[list(range(NCORES))],
                    ins=[a2a_in[:].opt()], outs=[a2a_out[:].opt()])
                a2a_bufs[(b, h)] = a2a_out

            def do_gather(b, h):
                # deferred until just before the consuming outproj tiles so no
                # earlier-emitted reader of gatT serializes behind it.
                a2a_out = a2a_bufs[(b, h)]
                rb = (b * 2 + h) * GRAN
                engs = [nc.sync, nc.scalar, nc.gpsimd]
                for c in range(DCH):
                    eng = engs[c % len(engs)]
                    eng.dma_start(out=gatT[:, c * NQ + rb:c * NQ + rb + GRAN],
                                  in_=a2a_out[c * P:(c + 1) * P, :])

            def outproj_ot(b, t, ot):
                rb = (b * 2 + t) * GRAN
                yp = midp.tile([P, IBLK], F32, tag="mid", name=f"yp{b}_{t}_{ot}")
                for kc in range(DCH):
                    nc.tensor.matmul(
                        yp[:], gatT[:, kc * NQ + rb:kc * NQ + rb + GRAN],
                        wo_sb[:, kc * DIM + ot * IBLK:kc * DIM + (ot + 1) * IBLK],
                        start=(kc == 0), stop=(kc == DCH - 1))
                yo = ypool.tile([P, IBLK], F32, tag="yo", name=f"yo{b}_{t}_{ot}")
                nc.vector.tensor_add(yo[:], yp[:], bor_sb[:, ot * IBLK:(ot + 1) * IBLK])
                nc.sync.dma_start(
                    out=out[rb:rb + GRAN, ot * IBLK:(ot + 1) * IBLK], in_=yo[:])

            def cunits(b, t):
                def first():
                    do_gather(b, t)
                    outproj_ot(b, t, 0)
                return [first] + [lambda ot=ot: outproj_ot(b, t, ot)
                                  for ot in range(1, DIM // IBLK)]

            # emission order: i-blocks software-pipelined (QK of ib with av of
            # ib-1); exchanges fire as soon as their half's attT is emitted;
            # b0's outproj ot-tiles slot into b1's attention as extra units.
            # outproj(1,0) runs after the last exchange is triggered, filling
            # the wait for the final collective.
            emit_ib(0, 0, None)
            emit_ib(0, 1, (0, 0))
            emit_ib(0, 2, (0, 1))
            do_exchange(0, 0)
            emit_ib(0, 3, (0, 2))
            emit_ib(1, 0, (0, 3))
            do_exchange(0, 1)
            emit_ib(1, 1, (1, 0))
            emit_ib(1, 2, (1, 1), extra=cunits(0, 0))
            do_exchange(1, 0)
            emit_ib(1, 3, (1, 2), extra=cunits(0, 1))
            flush_ib(1, 3)
            do_exchange(1, 1)
            for u in cunits(1, 0):
                u()
            for u in cunits(1, 1):
                u()


def build_nc():
    nc = bacc_mod.Bacc(None, target_bir_lowering=False, debug=False)
    xT = nc.declare_dram_parameter("xT", [DIM, NT], BF16, isOutput=False)
    wqk = nc.declare_dram_parameter("wqk", [P, DCH * P], BF16, isOutput=False)
    wv = nc.declare_dram_parameter("wv", [P, DCH * (DV + 1)], BF16, isOutput=False)
    wo = nc.declare_dram_parameter("wo", [P, DCH * DIM], BF16, isOutput=False)
    krows = nc.declare_dram_parameter("krows", [3, N], BF16, isOutput=False)
    ccol = nc.declare_dram_parameter("ccol", [P, 2], F32, isOutput=False)
    bor = nc.declare_dram_parameter("bor", [P, DIM], F32, isOutput=False)
    out = nc.declare_dram_parameter("out", [NQ, DIM], F32, isOutput=True)
    with tile.TileContext(nc) as tc:
        _build_body(nc, tc, xT, wqk, wv, wo, krows, ccol, bor, out)
    nc.compile()
    return nc


def _in_maps(x, Wq, Wk, Wv, W_rel, Wo, bo, rcb, rpb):
    scale = np.float32(DK ** -0.5)
    Wq_s = (Wq * scale).astype(np.float32)
    iota = np.arange(N, dtype=np.float32)
    jhi = np.floor(iota / 8)
    jlo = iota - 8 * jhi
    krows = np.stack([jhi, jlo, np.ones(N, np.float32)]).astype(ml_dtypes.bfloat16)
    bor = np.broadcast_to(bo.astype(np.float32), (P, DIM)).copy()
    xTb = np.ascontiguousarray(np.concatenate([x[0].T, x[1].T], axis=1)).astype(ml_dtypes.bfloat16)

    def chunk(w):  # [DIM, M] -> [P, DCH*M] (d-chunk-major)
        m = w.shape[1]
        return np.ascontiguousarray(
            w.reshape(DCH, P, m).transpose(1, 0, 2).reshape(P, DCH * m)).astype(ml_dtypes.bfloat16)

    wo_ch = chunk(Wo)

    maps = []
    for h in range(NCORES):
        qs, ks = Wq_s[:, h * DK:(h + 1) * DK], Wk[:, h * DK:(h + 1) * DK]
        vs = Wv[:, h * DV:(h + 1) * DV]
        w_h = W_rel[0, h * DK:(h + 1) * DK]
        u = (qs @ w_h)[:, None]          # s per token = x @ u (+ rpb@w_h)
        corr = np.float32(rpb[h] @ w_h)
        ccol = np.zeros((P, 2), np.float32)
        ccol[0:DK, 0] = rcb[h]
        ccol[:, 1] = corr
        maps.append({
            "xT": xTb,
            "wqk": chunk(np.concatenate([qs, ks], axis=1)),
            "wv": chunk(np.concatenate([vs, u], axis=1)),
            "wo": wo_ch,
            "krows": krows,
            "ccol": ccol,
            "bor": bor,
        })
    return maps


def kernel(x, Wq, Wk, Wv, W_rel, Wo, bo, rel_content_bias, rel_pos_bias):
    x = np.asarray(x, np.float32)
    rcb = np.asarray(rel_content_bias, np.float32)[0, :, 0, :]
    rpb = np.asarray(rel_pos_bias, np.float32)[0, :, 0, :]
    if "nc" not in _CACHE:
        _CACHE["nc"] = build_nc()
    nc = _CACHE["nc"]
    maps = _in_maps(x, np.asarray(Wq, np.float32), np.asarray(Wk, np.float32),
                    np.asarray(Wv, np.float32), np.asarray(W_rel, np.float32),
                    np.asarray(Wo, np.float32), np.asarray(bo, np.float32), rcb, rpb)
    # first execution pays NEFF-load / launch-skew costs on all 8 cores;
    # run once to warm, then execute for real.
    run_bass_kernel_spmd(nc, maps, core_ids=list(range(NCORES)))
    res = run_bass_kernel_spmd(nc, maps, core_ids=list(range(NCORES)))
    out = np.zeros((B, N, DIM), np.float32)
    for c in range(NCORES):
        o = res.results[c]["out"]
        for b in range(B):
            for h in range(2):
                out[b, h * 1024 + c * GRAN:h * 1024 + (c + 1) * GRAN, :] = \
                    o[(b * 2 + h) * GRAN:(b * 2 + h + 1) * GRAN]
    return out.reshape(B, N, DIM)
